# revision 33
# baseline (speedup 1.0000x reference)
"""Longformer sliding-chunk attention (B=2, S=4096, E=1024, H=16, W=256) on 8 trn2 cores.

Sharding: tensor-parallel over heads — core c owns heads {2c, 2c+1}. Each core:
  - projects q/k for its 128 output features (2 heads x 64) over the full
    [8192, 1024] hidden states in transposed [d, s] layout (lhsT = weights)
  - projects v in [s, d] layout directly (lhsT = hidden-state tile, rhs = Wv^T),
    so no PE transposes are needed; v bias is folded in on the host
  - computes chunked attention fully transposed: scoresT = K @ Q^T per
    128-key-block into a [128, n_kb, 256] PSUM strip, one wide exp ACTIVATE
    per (chunk, head) (no max subtraction: scores are O(1) here), probsT @ V
    via PE with an appended ones-column that yields softmax denominators free
  - ships unnormalized numerator^T + denominators as [130, 8192]:
    rows 0:64 head0 numerator, 64 head0 denom, 65:129 head1 num, 129 h1 denom
Host adds boundary-mask pad mass to denominators, normalizes, and adds bv.

All matmuls run in bfloat16 (fp32 PSUM accumulate); projections+attention are
interleaved per 1024-token tile so exp/bias ACTIVATEs hide under PE matmuls.
"""
import numpy as np
import ml_dtypes

import concourse.bass as bass
import concourse.mybir as mybir
import concourse.tile as tile
from concourse import bacc
from concourse.bass_utils import run_bass_kernel_spmd

F32 = mybir.dt.float32
BF16 = mybir.dt.bfloat16
AFT = mybir.ActivationFunctionType

B, S, E = 2, 4096, 1024
H, W, D = 16, 256, 64
BS = B * S           # 8192
NT = 4               # 1024-wide seq tiles per batch
KT = 8               # contraction tiles of 128 over E
NCHUNK = S // W      # 16 chunks per batch
NKB = S // 128       # 32 key blocks of 128 per batch

_NC_CACHE = None
_BV = None           # host-side v bias, folded in during assemble


def _build():
    nc = bacc.Bacc("TRN2", target_bir_lowering=False, debug=False, num_devices=8)

    # host ships hidden states pre-tiled: [128, tile(8), kt(8), seq(1024)] so
    # one projection tile is a single 128-descriptor DMA of 16 KB/descriptor
    hsT = nc.dram_tensor("hsT", [128, (B * S // 1024) * KT * 1024], BF16,
                         kind="ExternalInput").ap()
    w_ap = {}
    for nm in ("q", "k", "v"):
        # host ships weights pre-arranged as [128, KT*128]: row p holds
        # W^T[kt*128+p, :] for kt = 0..7 -> one 2 KB descriptor per partition
        w_ap[nm] = nc.dram_tensor(f"w{nm}T", [128, KT * 128], BF16, kind="ExternalInput").ap()
    b_ap = {
        nm: nc.dram_tensor(f"b{nm}", [128, 1], F32, kind="ExternalInput").ap()
        for nm in ("q", "k")
    }
    outT = nc.dram_tensor("outT", [130, BS], F32, kind="ExternalOutput").ap()

    with tile.TileContext(nc) as tc:
        with (
            tc.tile_pool(name="singles", bufs=1) as singles,
            tc.tile_pool(name="big", bufs=1) as big,
            tc.tile_pool(name="hst", bufs=3) as hpool,
            tc.tile_pool(name="probs", bufs=4) as probs_pool,
            tc.tile_pool(name="stage", bufs=4) as stage_pool,
            tc.tile_pool(name="pssm", bufs=2, space="PSUM") as ps_sm,
            tc.tile_pool(name="pssc", bufs=2, space="PSUM") as ps_sc,
        ):
            hsT_r = hsT.rearrange("p (t kt s) -> p t kt s", kt=KT, s=1024)

            # Tiny first DMA so the DMA queues spin up during the ~5us
            # engine preamble instead of delaying the first weight load.
            warm = singles.tile([128, 16], BF16, tag="warm")
            nc.sync.dma_start(out=warm, in_=w_ap["q"][:, 0:16])

            # DMA issue order tuned for startup: q/k weights first, then the
            # first hidden-state tile, then biases/wv/ones.
            w_sb = {}
            b_sb = {}
            for nm in ("q", "k"):
                wt = singles.tile([128, KT, 128], BF16, tag=f"w{nm}")
                nc.sync.dma_start(
                    out=wt, in_=w_ap[nm].rearrange("p (kt m) -> p kt m", m=128)
                )
                w_sb[nm] = wt

            hst_pre = {}

            def load_hst(b, n):
                t = b * NT + n
                hst = hpool.tile([128, KT, 1024], BF16, tag="hst")
                # one DMA per contraction slab so matmul k can start as soon
                # as slab k lands
                for k in range(KT):
                    nc.sync.dma_start(
                        out=hst[:, k : k + 1, :], in_=hsT_r[:, t, k : k + 1, :]
                    )
                return hst

            hst_pre[(0, 0)] = load_hst(0, 0)

            for nm in ("q", "k"):
                bt = singles.tile([128, 1], F32, tag=f"b{nm}")
                nc.sync.dma_start(out=bt, in_=b_ap[nm])
                b_sb[nm] = bt
            wt = singles.tile([128, KT, 128], BF16, tag="wv")
            nc.sync.dma_start(
                out=wt, in_=w_ap["v"].rearrange("p (kt m) -> p kt m", m=128)
            )
            w_sb["v"] = wt

            QT = big.tile([128, BS], BF16, tag="qt")
            KTt = big.tile([128, BS], BF16, tag="kt")
            # v store: per 128-key block, [seq 128, 65*2]: cols 0:64 head0 d,
            # 64 ones, 65:129 head1 d, 129 ones
            vfull = big.tile([128, B * NKB, 130], BF16, tag="vfull")
            nc.vector.memset(
                vfull.rearrange("p s (x o) -> p s x o", x=2)[:, :, :, 64:65], 1.0
            )

            def attn_scores(b, c):
                """Issue score matmuls + exp for chunk c; return PV context."""
                base = b * S
                lo = max(0, 2 * c - 2)
                hi = min(NKB, 2 * c + 4)
                n_kb = hi - lo
                q_sl = slice(base + c * W, base + (c + 1) * W)
                prs = {}
                for h in (0, 1):
                    d_sl = slice(h * 64, (h + 1) * 64)
                    sps = ps_sc.tile([128, 6, 256], F32, tag="sc", name=f"s{h}_{b}_{c}")
                    for i in range(n_kb):
                        kb = lo + i
                        k_sl = slice(base + kb * 128, base + (kb + 1) * 128)
                        nc.tensor.matmul(
                            sps[:, i, :],
                            lhsT=KTt[d_sl, k_sl],
                            rhs=QT[d_sl, q_sl],
                            start=True,
                            stop=True,
                        )
                    pr = probs_pool.tile(
                        [128, 6, 256], BF16, tag="probs", name=f"pr{h}_{b}_{c}"
                    )
                    nc.scalar.activation(
                        pr[:, 0:n_kb, :], sps[:, 0:n_kb, :], AFT.Exp
                    )
                    prs[h] = pr
                return (b, c, lo, n_kb, q_sl, prs)

            def attn_pv(ctx):
                b, c, lo, n_kb, q_sl, prs = ctx
                for h in (0, 1):
                    po = ps_sm.tile([65, 256], F32, tag="sm", name=f"pv{h}_{b}_{c}")
                    for i in range(n_kb):
                        slot = b * NKB + lo + i
                        nc.tensor.matmul(
                            po,
                            lhsT=vfull[:, slot, h * 65 : (h + 1) * 65],
                            rhs=prs[h][:, i, :],
                            start=(i == 0),
                            stop=(i == n_kb - 1),
                        )
                    st = stage_pool.tile([65, 256], F32, tag="stage")
                    nc.vector.tensor_copy(st, po)
                    nc.sync.dma_start(
                        out=outT[h * 65 : (h + 1) * 65, q_sl], in_=st
                    )

            pending = None
            for b in range(B):
                base = b * S
                c_done = 0
                for n in range(NT):
                    hst = hst_pre.pop((b, n), None)
                    if hst is None:
                        hst = load_hst(b, n)
                    # q/k projections: [feat, seq] layout, lhsT = weights
                    for nm, dest, scale, bias in (
                        ("q", QT, 1.0 / np.sqrt(D), b_sb["q"]),
                        ("k", KTt, 1.0, b_sb["k"]),
                    ):
                        for half in (0, 1):
                            hsl = slice(half * 512, (half + 1) * 512)
                            osl = slice(
                                base + n * 1024 + half * 512,
                                base + n * 1024 + (half + 1) * 512,
                            )
                            psp = ps_sm.tile(
                                [128, 512], F32, tag="sm", name=f"p{nm}_{b}_{n}_{half}"
                            )
                            for k in range(KT):
                                nc.tensor.matmul(
                                    psp,
                                    lhsT=w_sb[nm][:, k, :],
                                    rhs=hst[:, k, hsl],
                                    start=(k == 0),
                                    stop=(k == KT - 1),
                                )
                            nc.scalar.activation(
                                dest[:, osl], psp, AFT.Identity, bias=bias, scale=scale
                            )
                    # v projection: [seq, feat] layout, lhsT = hidden tile
                    for j in range(KT):
                        kb = n * 8 + j
                        ssl = slice(j * 128, (j + 1) * 128)
                        psv = ps_sm.tile(
                            [128, 128], F32, tag="sm", name=f"pv_{b}_{n}_{j}"
                        )
                        for k in range(KT):
                            nc.tensor.matmul(
                                psv,
                                lhsT=hst[:, k, ssl],
                                rhs=w_sb["v"][:, k, :],
                                start=(k == 0),
                                stop=(k == KT - 1),
                            )
                        slot = b * NKB + kb
                        nc.vector.tensor_copy(
                            vfull[:, slot, 0:130].rearrange("p (x o) -> p x o", x=2)[
                                :, :, 0:64
                            ],
                            psv.rearrange("p (x o) -> p x o", x=2),
                        )
                    # attention chunks whose window is now fully projected
                    c_hi = NCHUNK if n == NT - 1 else 4 * n + 3
                    for c in range(c_done, c_hi):
                        ctx = attn_scores(b, c)
                        if pending is not None:
                            attn_pv(pending)
                        pending = ctx
                    c_done = c_hi
            if pending is not None:
                attn_pv(pending)

    nc.compile()
    return nc


def get_nc():
    global _NC_CACHE
    if _NC_CACHE is None:
        _NC_CACHE = _build()
    return _NC_CACHE


def _w_prep(w):
    """[128, E] slice of W -> [128 partition, KT*128] with row p holding
    W^T[kt*128+p, :] for kt = 0..KT-1."""
    bf = ml_dtypes.bfloat16
    wt = w.T.reshape(KT, 128, 128)  # [kt, p, m]
    return np.ascontiguousarray(wt.transpose(1, 0, 2).reshape(128, KT * 128)).astype(bf)


def make_in_maps(hidden_states, Wq, bq, Wk, bk, Wv, bv):
    global _BV
    _BV = np.asarray(bv, np.float32)
    bf = ml_dtypes.bfloat16
    # [E, BS] -> [128(p), tile(8), kt(8), 1024] -> [128, 64K], row-contiguous
    hsTf = hidden_states.reshape(BS, E).T  # [E, BS]
    hsT = np.ascontiguousarray(
        hsTf.reshape(KT, 128, B * NT, 1024).transpose(1, 2, 0, 3).reshape(128, -1)
    ).astype(bf)
    in_maps = []
    for c in range(8):
        fsl = slice(c * 128, (c + 1) * 128)
        in_maps.append(
            {
                "hsT": hsT,
                "wqT": _w_prep(Wq[fsl]),
                "wkT": _w_prep(Wk[fsl]),
                "wvT": _w_prep(Wv[fsl]),
                "bq": np.ascontiguousarray(
                    (bq[fsl] / np.sqrt(D)).reshape(128, 1).astype(np.float32)
                ),
                "bk": np.ascontiguousarray(bk[fsl].reshape(128, 1).astype(np.float32)),
            }
        )
    return in_maps


def assemble(results):
    """results: list of 8 per-core dicts with 'outT' [130, BS] -> full [B,S,E]."""
    # boundary pad mass: chunk 0 row ii has ii unmasked zero-score pad keys,
    # chunk 15 row ii has 255-ii
    pad = np.zeros(S, np.float32)
    pad[:W] = np.arange(W, dtype=np.float32)
    pad[S - W :] = (W - 1) - np.arange(W, dtype=np.float32)
    pad_bs = np.tile(pad, B)  # [BS]

    out = np.empty((B, S, E), np.float32)
    for c in range(8):
        oT = np.asarray(results[c]["outT"], np.float32)  # [130, BS]
        for hl in (0, 1):
            num = oT[hl * 65 : hl * 65 + 64].T  # [BS, 64]
            den_dev = oT[hl * 65 + 64]  # [BS]
            den = den_dev + pad_bs
            fsl = slice(c * 128 + hl * 64, c * 128 + (hl + 1) * 64)
            full = (num + den_dev[:, None] * _BV[None, fsl]) / den[:, None]
            out.reshape(BS, E)[:, fsl] = full
    return out


def kernel(hidden_states, Wq, bq, Wk, bk, Wv, bv):
    hidden_states, Wq, bq, Wk, bk, Wv, bv = (
        np.asarray(x, np.float32) for x in (hidden_states, Wq, bq, Wk, bk, Wv, bv)
    )
    nc = get_nc()
    in_maps = make_in_maps(hidden_states, Wq, bq, Wk, bk, Wv, bv)
    last_exc = None
    for attempt in range(3):
        try:
            res = run_bass_kernel_spmd(nc, in_maps, list(range(8)))
            return assemble(res.results)
        except Exception as exc:  # transient device hiccups (e.g. NRT exec)
            last_exc = exc
            import time

            time.sleep(2.0)
    raise last_exc
